# revision 1
# baseline (speedup 1.0000x reference)
"""2-layer weighted-GCN embedding kernel for 8 Trainium2 NeuronCores.

Strategy (dst-sharded message passing):
  - Nodes are sharded by destination across the 8 cores (12500 each, padded
    to 12544 = 98 * 128).  Each core handles every edge whose dst lands in
    its shard, so the scatter-add is purely local.
  - GCN associativity: conv(x) = (A_hat @ x) @ W^T + b, so we aggregate RAW
    features first and apply the dense transform on the (sharded) aggregate.
  - Per-edge gather of source rows uses the SWDGE dma_gather instruction
    (bf16 rows, 256 B each).  Indices are int16, so the padded node table
    (100352 rows) is split into 4 chunks of 25088 rows.
  - Scatter-add is an indicator matmul: for each block of 128 edges, DVE
    builds ind[e, j] = (dst_rel[e] == j) * w[e] and the tensor engine
    accumulates ind^T @ msg into the PSUM tile of the 128-node dst subtile.
  - Between the two conv layers one AllGather shares the hidden state
    r1' = dinv * relu(conv1) across cores (bf16).
  - Normalization folded in: gather source is xp = dinv * x, indicator
    carries the raw edge weight, and the remaining dinv[dst] factor rides
    the transpose matmul via a diag(dinv) stationary operand.

kernel(**inputs) takes the FULL inputs and returns the FULL [100000, 64]
output; everything (sharding, compile, SPMD run, gather of shards) happens
inside.
"""

import numpy as np
import ml_dtypes

import concourse.bass as bass
import concourse.tile as tile
import concourse.bacc as bacc
from concourse import mybir, bass_utils

BF16 = ml_dtypes.bfloat16

F = 128
HID = 128
ENC = 64
NCORES = 8
SUBW = 128
SUPSZ = 6                      # subtiles per supertile (one gather covers these)


def _set_dims(n):
    """(Re)compute the node-count-derived global dims. Called at import with
    the real N; tests may call with a tiny N."""
    global N, SHARD, NSUB, SHARD_PAD, CHUNK, XROWS, NSUP
    N = n
    SHARD = N // NCORES
    NSUB = -(-SHARD // SUBW)           # subtiles per shard
    SHARD_PAD = NSUB * SUBW
    CHUNK = 2 * SHARD_PAD              # rows per gather chunk (< 2**15)
    XROWS = NCORES * SHARD_PAD         # padded node-table rows
    NSUP = -(-NSUB // SUPSZ)


NCHUNK = 4
_set_dims(100000)

_cache = {}


def _preprocess(x, edge_index, edge_weight, W1, b1, W2, b2, Wf, bf):
    """All host-side numpy prep: normalization, edge partitioning, layouts."""
    src = np.asarray(edge_index[0], dtype=np.int64)
    dst = np.asarray(edge_index[1], dtype=np.int64)
    w = np.asarray(edge_weight, dtype=np.float32)
    x = np.asarray(x, dtype=np.float32)

    deg = np.bincount(dst, weights=w.astype(np.float64), minlength=N) + 1.0
    dinv = (1.0 / np.sqrt(deg)).astype(np.float32)

    xp = x * dinv[:, None]
    xp_pad = np.zeros((XROWS, F), np.float32)
    for o in range(NCORES):
        xp_pad[o * SHARD_PAD:o * SHARD_PAD + SHARD] = xp[o * SHARD:(o + 1) * SHARD]
    xp_bf = xp_pad.astype(BF16)

    # map src node id -> (chunk, local row) in the padded table
    owner = src // SHARD
    src_pad = owner * SHARD_PAD + (src - owner * SHARD)
    chunk = src_pad // CHUNK
    src_local = (src_pad - chunk * CHUNK).astype(np.int64)
    assert src_local.max() < 2 ** 15

    NCELL = NCHUNK * NSUB  # flat cell id = c * NSUB + t

    # per-device cell contents
    dev = []
    counts = np.zeros((NCORES, NCELL), np.int64)
    for d in range(NCORES):
        lo, hi = d * SHARD, (d + 1) * SHARD
        m = (dst >= lo) & (dst < hi)
        dl = dst[m] - lo
        t = dl // SUBW
        cid = chunk[m] * NSUB + t
        order = np.argsort(cid, kind="stable")
        cid_s = cid[order]
        counts[d] = np.bincount(cid_s, minlength=NCELL)
        dev.append((cid_s,
                    src_local[m][order].astype(np.int16),
                    (dl % SUBW)[order].astype(np.float32),
                    w[m][order]))

    nb_cell = -(-counts.max(axis=0) // 128)            # blocks per cell (shared)
    cell_off = np.zeros(NCELL + 1, np.int64)
    np.cumsum(nb_cell * 128, out=cell_off[1:])
    TOT = int(cell_off[-1])

    per_core = []
    for d in range(NCORES):
        cid_s, sl, dr, wl = dev[d]
        starts = np.zeros(NCELL + 1, np.int64)
        np.cumsum(counts[d], out=starts[1:])
        rank = np.arange(len(cid_s)) - starts[cid_s]
        pos = cell_off[cid_s] + rank
        f_src = np.zeros(TOT, np.int16)
        f_dr = np.zeros(TOT, np.float32)
        f_w = np.zeros(TOT, np.float32)
        f_src[pos] = sl
        f_dr[pos] = dr
        f_w[pos] = wl

        idx16 = np.ascontiguousarray(np.tile(f_src.reshape(-1, 16).T, (8, 1)))
        # host-built indicators, partition-major: indb[p, blk*128 + dst_rel] = w
        # (slot = blk*128 + p; one matmul block = columns [blk*128,(blk+1)*128))
        indb = np.zeros((128, TOT), BF16)
        pos = np.arange(TOT)
        indb[pos % 128, (pos // 128) * 128 + f_dr.astype(np.int64)] = \
            f_w.astype(BF16)

        lo = d * SHARD
        dvt = np.ones(SHARD_PAD, np.float32)
        dvt[:SHARD] = dinv[lo:lo + SHARD]
        dinv_t = np.ascontiguousarray(dvt.reshape(NSUB, SUBW).T)  # [128, NSUB]

        per_core.append({
            "idx16": idx16,
            "indb": indb,
            "dinv_t": dinv_t,
            "xp_self": xp_pad[d * SHARD_PAD:(d + 1) * SHARD_PAD].copy(),
        })

    shared = {
        "xp_bf": xp_bf,
        "w1t": np.ascontiguousarray(np.asarray(W1, np.float32).T),
        "w2t": np.ascontiguousarray(np.asarray(W2, np.float32).T),
        "wft": np.ascontiguousarray(np.asarray(Wf, np.float32).T),
        "b1bc": np.broadcast_to(np.asarray(b1, np.float32), (128, HID)).copy(),
        "b2bc": np.broadcast_to(np.asarray(b2, np.float32), (128, HID)).copy(),
        "bfbc": np.broadcast_to(np.asarray(bf, np.float32), (128, ENC)).copy(),
        "ident": np.eye(128, dtype=np.float32),
    }
    nb = nb_cell.reshape(NCHUNK, NSUB)      # [c][t]
    offs = cell_off.reshape(-1)             # flat slot offsets, id = c*NSUB+t
    return shared, per_core, nb, offs, TOT


def _build(nb, offs, TOT, stage=3):
    """Build the SPMD bass program (identical for all 8 cores).

    stage: 1 = layer-1 aggregation only, 2 = + collective, 3 = full."""
    nc = bacc.Bacc("TRN2", target_bir_lowering=False, debug=False,
                   num_devices=NCORES)
    f32 = mybir.dt.float32
    bf16 = mybir.dt.bfloat16

    xp_bf_t = nc.dram_tensor("xp_bf", [XROWS, F], bf16, kind="ExternalInput")
    xp_self_t = nc.dram_tensor("xp_self", [SHARD_PAD, F], f32, kind="ExternalInput")
    idx16_t = nc.dram_tensor("idx16", [128, TOT // 16], mybir.dt.int16, kind="ExternalInput")
    indb_t = nc.dram_tensor("indb", [128, TOT], bf16, kind="ExternalInput")
    dinv_t_t = nc.dram_tensor("dinv_t", [128, NSUB], f32, kind="ExternalInput")
    w1t_t = nc.dram_tensor("w1t", [F, HID], f32, kind="ExternalInput")
    w2t_t = nc.dram_tensor("w2t", [HID, HID], f32, kind="ExternalInput")
    wft_t = nc.dram_tensor("wft", [HID, ENC], f32, kind="ExternalInput")
    b1bc_t = nc.dram_tensor("b1bc", [128, HID], f32, kind="ExternalInput")
    b2bc_t = nc.dram_tensor("b2bc", [128, HID], f32, kind="ExternalInput")
    bfbc_t = nc.dram_tensor("bfbc", [128, ENC], f32, kind="ExternalInput")
    ident_t = nc.dram_tensor("ident", [128, 128], f32, kind="ExternalInput")
    out_t = nc.dram_tensor("out", [SHARD_PAD, ENC], f32, kind="ExternalOutput")

    # per-subtile block lists: blocks[t] = ordered [(c, k), ...]
    blocks = [[(c, k) for c in range(NCHUNK) for k in range(int(nb[c][t]))]
              for t in range(NSUB)]

    with tile.TileContext(nc) as tc:
        with tc.tile_pool(name="const", bufs=1) as cst, \
             tc.tile_pool(name="edata", bufs=1) as edata, \
             tc.tile_pool(name="msgp", bufs=2) as msgp, \
             tc.tile_pool(name="indp", bufs=4) as indp, \
             tc.tile_pool(name="accp", bufs=3, space="PSUM") as accp, \
             tc.tile_pool(name="epsp", bufs=3, space="PSUM") as epsp, \
             tc.tile_pool(name="work", bufs=3) as work, \
             tc.tile_pool(name="dram", bufs=1, space="DRAM") as dram:

            # ---- persistent SBUF data ----
            idx_sb = edata.tile([128, TOT // 16], mybir.dt.int16)
            nc.sync.dma_start(idx_sb[:], idx16_t[:])

            dinv_sb = cst.tile([128, NSUB], f32)
            w1t_sb = cst.tile([F, HID], f32)
            w2t_sb = cst.tile([HID, HID], f32)
            wft_sb = cst.tile([HID, ENC], f32)
            b1bc_sb = cst.tile([128, HID], f32)
            b2bc_sb = cst.tile([128, HID], f32)
            bfbc_sb = cst.tile([128, ENC], f32)
            ident_sb = cst.tile([128, 128], f32)
            for sb_, t_ in ((dinv_sb, dinv_t_t), (w1t_sb, w1t_t), (w2t_sb, w2t_t),
                            (wft_sb, wft_t), (b1bc_sb, b1bc_t), (b2bc_sb, b2bc_t),
                            (bfbc_sb, bfbc_t), (ident_sb, ident_t)):
                nc.sync.dma_start(sb_[:], t_[:])

            r1self_sb = edata.tile([128, NSUB * HID], f32)

            r1sh = dram.tile([SHARD_PAD, HID], bf16)
            r1full = dram.tile([XROWS, HID], bf16, addr_space="Shared")

            def aggregate_layer(src_dram, layer):
                """Gather + indicator-matmul aggregation + per-subtile epilogue.

                Block order is subtile-major so each subtile's PSUM
                accumulation group opens and closes before the next one
                starts (accumulation groups are bank-granular)."""
                for s in range(NSUP):
                    subs = list(range(s * SUPSZ, min((s + 1) * SUPSZ, NSUB)))
                    msgs = {}
                    starts = {}
                    for c in range(NCHUNK):
                        start_slot = int(offs[c * NSUB + subs[0]])
                        end_slot = int(offs[c * NSUB + subs[-1] + 1])
                        L = end_slot - start_slot
                        if L == 0:
                            continue
                        starts[c] = start_slot
                        msg = msgp.tile([128, L], bf16, tag=f"msg{c}", bufs=2)
                        msgs[c] = msg
                        nc.gpsimd.dma_gather(
                            msg[:].rearrange("p (b f) -> p b f", f=128),
                            src_dram[c * CHUNK:(c + 1) * CHUNK, :],
                            idx_sb[:, start_slot // 16:end_slot // 16],
                            L, L, 128, elem_step=F,
                            single_packet=False,
                        )

                    # ---- per-subtile accumulate + drain ----
                    for t in subs:
                        acc = accp.tile([128, 128], f32, tag="acc")
                        for c in range(NCHUNK):
                            nbk = int(nb[c][t])
                            if nbk == 0:
                                continue
                            base = int(offs[c * NSUB + t])
                            ind = indp.tile([128, nbk * 128], bf16, tag="ind")
                            nc.scalar.dma_start(
                                ind[:], indb_t[:, base:base + nbk * 128])
                            for k in range(nbk):
                                mloc = (base - starts[c]) // 128 + k
                                nc.tensor.matmul(
                                    acc[:],
                                    lhsT=ind[:, k * 128:(k + 1) * 128],
                                    rhs=msgs[c][:, mloc * 128:(mloc + 1) * 128],
                                    start=(blocks[t][0] == (c, k)),
                                    stop=(blocks[t][-1] == (c, k)),
                                )

                        sum_sb = work.tile([128, F], f32, tag="sum")
                        if layer == 0:
                            self_tl = work.tile([128, F], f32, tag="selftl")
                            nc.sync.dma_start(
                                self_tl[:], xp_self_t[t * 128:(t + 1) * 128, :])
                        else:
                            self_tl = r1self_sb[:, t * HID:(t + 1) * HID]
                        if blocks[t]:
                            nc.vector.tensor_tensor(
                                out=sum_sb[:], in0=acc[:], in1=self_tl[:],
                                op=mybir.AluOpType.add)
                        else:
                            nc.vector.tensor_copy(out=sum_sb[:], in_=self_tl[:])

                        diag = work.tile([128, 128], f32, tag="diag")
                        nc.scalar.activation(
                            diag[:], ident_sb[:],
                            mybir.ActivationFunctionType.Copy,
                            scale=dinv_sb[:, t:t + 1])
                        tp = epsp.tile([128, 128], f32, tag="eps")
                        nc.tensor.matmul(tp[:], lhsT=sum_sb[:], rhs=diag[:],
                                         start=True, stop=True)
                        ts = work.tile([128, 128], f32, tag="ts")
                        nc.scalar.activation(ts[:], tp[:],
                                             mybir.ActivationFunctionType.Copy)

                        wsb = w1t_sb if layer == 0 else w2t_sb
                        op_ = epsp.tile([128, HID], f32, tag="eps")
                        nc.tensor.matmul(op_[:], lhsT=ts[:], rhs=wsb[:],
                                         start=True, stop=True)
                        z = work.tile([128, HID], f32, tag="z")
                        bbc = b1bc_sb if layer == 0 else b2bc_sb
                        nc.vector.tensor_tensor(out=z[:], in0=op_[:], in1=bbc[:],
                                                op=mybir.AluOpType.add)

                        if layer == 0:
                            # r1' = relu(z * dinv) kept f32 in SBUF + bf16 to HBM
                            nc.scalar.activation(
                                r1self_sb[:, t * HID:(t + 1) * HID], z[:],
                                mybir.ActivationFunctionType.Relu,
                                scale=dinv_sb[:, t:t + 1])
                            r1bf = work.tile([128, HID], bf16, tag="r1bf")
                            nc.scalar.activation(
                                r1bf[:], z[:],
                                mybir.ActivationFunctionType.Relu,
                                scale=dinv_sb[:, t:t + 1])
                            nc.sync.dma_start(
                                r1sh[t * 128:(t + 1) * 128, :], r1bf[:])
                        else:
                            r2 = work.tile([128, HID], f32, tag="r2")
                            nc.scalar.activation(
                                r2[:], z[:], mybir.ActivationFunctionType.Relu)
                            rtp = epsp.tile([128, 128], f32, tag="eps")
                            nc.tensor.matmul(rtp[:], lhsT=r2[:], rhs=ident_sb[:],
                                             start=True, stop=True)
                            rts = work.tile([128, 128], f32, tag="rts")
                            nc.scalar.activation(rts[:], rtp[:],
                                                 mybir.ActivationFunctionType.Copy)
                            fp = epsp.tile([128, ENC], f32, tag="eps")
                            nc.tensor.matmul(fp[:], lhsT=rts[:], rhs=wft_sb[:],
                                             start=True, stop=True)
                            fz = work.tile([128, ENC], f32, tag="fz")
                            nc.vector.tensor_tensor(out=fz[:], in0=fp[:],
                                                    in1=bfbc_sb[:],
                                                    op=mybir.AluOpType.add)
                            nc.sync.dma_start(
                                out_t[t * 128:(t + 1) * 128, :], fz[:])

            aggregate_layer(xp_bf_t, layer=0)
            if stage >= 2:
                nc.gpsimd.collective_compute(
                    "AllGather",
                    mybir.AluOpType.bypass,
                    replica_groups=[list(range(NCORES))],
                    ins=[r1sh[:].opt()],
                    outs=[r1full[:].opt()],
                )
            if stage >= 3:
                aggregate_layer(r1full, layer=1)
            else:
                for t in range(NSUB):
                    dbg = work.tile([128, ENC], f32, tag="fz")
                    nc.vector.tensor_copy(
                        out=dbg[:], in_=r1self_sb[:, t * HID:t * HID + ENC])
                    nc.sync.dma_start(out_t[t * 128:(t + 1) * 128, :], dbg[:])

    nc.compile()
    return nc


def kernel(**inputs):
    shared, per_core, nb, offs, TOT = _preprocess(
        inputs["x"], inputs["edge_index"], inputs["edge_weight"],
        inputs["W1"], inputs["b1"], inputs["W2"], inputs["b2"],
        inputs["Wf"], inputs["bf"])

    key = (TOT, nb.tobytes())
    if key not in _cache:
        _cache[key] = _build(nb, offs, TOT)
    nc = _cache[key]

    in_maps = []
    for d in range(NCORES):
        m = dict(shared)
        m.update(per_core[d])
        in_maps.append(m)

    res = bass_utils.run_bass_kernel_spmd(nc, in_maps, core_ids=list(range(NCORES)))
    out = np.concatenate(
        [res.results[d]["out"][:SHARD] for d in range(NCORES)], axis=0)
    return out.astype(np.float32)



# revision 3
# speedup vs baseline: 1.6572x; 1.6572x over previous
"""2-layer weighted-GCN embedding kernel for 8 Trainium2 NeuronCores.

Strategy (dst-sharded message passing):
  - Nodes are sharded by destination across the 8 cores (12500 each, padded
    to 12544 = 98 * 128).  Each core handles every edge whose dst lands in
    its shard, so the scatter-add is purely local.
  - GCN associativity: conv(x) = (A_hat @ x) @ W^T + b, so we aggregate RAW
    features first and apply the dense transform on the (sharded) aggregate.
  - Per-edge gather of source rows uses the SWDGE dma_gather instruction
    (bf16 rows, 256 B each).  Indices are int16, so the padded node table
    (100352 rows) is split into 4 chunks of 25088 rows.
  - Scatter-add is an indicator matmul: for each block of 128 edges, DVE
    builds ind[e, j] = (dst_rel[e] == j) * w[e] and the tensor engine
    accumulates ind^T @ msg into the PSUM tile of the 128-node dst subtile.
  - Between the two conv layers one AllGather shares the hidden state
    r1' = dinv * relu(conv1) across cores (bf16).
  - Normalization folded in: gather source is xp = dinv * x, indicator
    carries the raw edge weight, and the remaining dinv[dst] factor rides
    the transpose matmul via a diag(dinv) stationary operand.

kernel(**inputs) takes the FULL inputs and returns the FULL [100000, 64]
output; everything (sharding, compile, SPMD run, gather of shards) happens
inside.
"""

import numpy as np
import ml_dtypes

import concourse.bass as bass
import concourse.tile as tile
import concourse.bacc as bacc
from concourse import mybir, bass_utils

BF16 = ml_dtypes.bfloat16

F = 128
HID = 128
ENC = 64
NCORES = 8
SUBW = 128
SUPSZ = 6                      # subtiles per supertile (one gather covers these)


def _set_dims(n):
    """(Re)compute the node-count-derived global dims. Called at import with
    the real N; tests may call with a tiny N."""
    global N, SHARD, NSUB, SHARD_PAD, CHUNK, XROWS, NSUP
    N = n
    SHARD = N // NCORES
    NSUB = -(-SHARD // SUBW)           # subtiles per shard
    SHARD_PAD = NSUB * SUBW
    CHUNK = 2 * SHARD_PAD              # rows per gather chunk (< 2**15)
    XROWS = NCORES * SHARD_PAD         # padded node-table rows
    NSUP = -(-NSUB // SUPSZ)


NCHUNK = 4
_set_dims(100000)

_cache = {}


def _preprocess(x, edge_index, edge_weight, W1, b1, W2, b2, Wf, bf):
    """All host-side numpy prep: normalization, edge partitioning, layouts."""
    src = np.asarray(edge_index[0], dtype=np.int64)
    dst = np.asarray(edge_index[1], dtype=np.int64)
    w = np.asarray(edge_weight, dtype=np.float32)
    x = np.asarray(x, dtype=np.float32)

    deg = np.bincount(dst, weights=w.astype(np.float64), minlength=N) + 1.0
    dinv = (1.0 / np.sqrt(deg)).astype(np.float32)

    xp = x * dinv[:, None]
    xp_pad = np.zeros((XROWS, F), np.float32)
    for o in range(NCORES):
        xp_pad[o * SHARD_PAD:o * SHARD_PAD + SHARD] = xp[o * SHARD:(o + 1) * SHARD]
    xp_bf = xp_pad.astype(BF16)

    # map src node id -> (chunk, local row) in the padded table
    owner = src // SHARD
    src_pad = owner * SHARD_PAD + (src - owner * SHARD)
    chunk = src_pad // CHUNK
    src_local = (src_pad - chunk * CHUNK).astype(np.int64)
    assert src_local.max() < 2 ** 15

    NCELL = NCHUNK * NSUB  # flat cell id = c * NSUB + t

    # per-device cell contents
    dev = []
    counts = np.zeros((NCORES, NCELL), np.int64)
    for d in range(NCORES):
        lo, hi = d * SHARD, (d + 1) * SHARD
        m = (dst >= lo) & (dst < hi)
        dl = dst[m] - lo
        t = dl // SUBW
        cid = chunk[m] * NSUB + t
        order = np.argsort(cid, kind="stable")
        cid_s = cid[order]
        counts[d] = np.bincount(cid_s, minlength=NCELL)
        dev.append((cid_s,
                    src_local[m][order].astype(np.int16),
                    (dl % SUBW)[order].astype(np.float32),
                    w[m][order]))

    nb_cell = -(-counts.max(axis=0) // 128)            # blocks per cell (shared)
    cell_off = np.zeros(NCELL + 1, np.int64)
    np.cumsum(nb_cell * 128, out=cell_off[1:])
    TOT = int(cell_off[-1])

    per_core = []
    for d in range(NCORES):
        cid_s, sl, dr, wl = dev[d]
        starts = np.zeros(NCELL + 1, np.int64)
        np.cumsum(counts[d], out=starts[1:])
        rank = np.arange(len(cid_s)) - starts[cid_s]
        pos = cell_off[cid_s] + rank
        f_src = np.zeros(TOT, np.int16)
        f_dr = np.zeros(TOT, np.float32)
        f_w = np.zeros(TOT, np.float32)
        f_src[pos] = sl
        f_dr[pos] = dr
        f_w[pos] = wl

        idx16 = np.ascontiguousarray(np.tile(f_src.reshape(-1, 16).T, (8, 1)))
        # host-built indicators, partition-major: indb[p, blk*128 + dst_rel] = w
        # (slot = blk*128 + p; one matmul block = columns [blk*128,(blk+1)*128))
        indb = np.zeros((128, TOT), BF16)
        pos = np.arange(TOT)
        indb[pos % 128, (pos // 128) * 128 + f_dr.astype(np.int64)] = \
            f_w.astype(BF16)

        lo = d * SHARD
        dvt = np.ones(SHARD_PAD, np.float32)
        dvt[:SHARD] = dinv[lo:lo + SHARD]
        dinv_t = np.ascontiguousarray(dvt.reshape(NSUB, SUBW).T)  # [128, NSUB]

        per_core.append({
            "idx16": idx16,
            "indb": indb,
            "dinv_t": dinv_t,
            "xp_self": xp_pad[d * SHARD_PAD:(d + 1) * SHARD_PAD].copy(),
        })

    shared = {
        "xp_bf": xp_bf,
        "w1t": np.ascontiguousarray(np.asarray(W1, np.float32).T),
        "w2t": np.ascontiguousarray(np.asarray(W2, np.float32).T),
        "wft": np.ascontiguousarray(np.asarray(Wf, np.float32).T),
        "b1bc": np.broadcast_to(np.asarray(b1, np.float32), (128, HID)).copy(),
        "b2bc": np.broadcast_to(np.asarray(b2, np.float32), (128, HID)).copy(),
        "bfbc": np.broadcast_to(np.asarray(bf, np.float32), (128, ENC)).copy(),
        "ident": np.eye(128, dtype=np.float32),
    }
    nb = nb_cell.reshape(NCHUNK, NSUB)      # [c][t]
    offs = cell_off.reshape(-1)             # flat slot offsets, id = c*NSUB+t
    return shared, per_core, nb, offs, TOT


def _build(nb, offs, TOT, stage=3):
    """Build the SPMD bass program (identical for all 8 cores).

    stage: 1 = layer-1 aggregation only, 2 = + collective, 3 = full."""
    nc = bacc.Bacc("TRN2", target_bir_lowering=False, debug=False,
                   num_devices=NCORES, num_swdge_queues=4)
    f32 = mybir.dt.float32
    bf16 = mybir.dt.bfloat16

    xp_bf_t = nc.dram_tensor("xp_bf", [XROWS, F], bf16, kind="ExternalInput")
    xp_self_t = nc.dram_tensor("xp_self", [SHARD_PAD, F], f32, kind="ExternalInput")
    idx16_t = nc.dram_tensor("idx16", [128, TOT // 16], mybir.dt.int16, kind="ExternalInput")
    indb_t = nc.dram_tensor("indb", [128, TOT], bf16, kind="ExternalInput")
    dinv_t_t = nc.dram_tensor("dinv_t", [128, NSUB], f32, kind="ExternalInput")
    w1t_t = nc.dram_tensor("w1t", [F, HID], f32, kind="ExternalInput")
    w2t_t = nc.dram_tensor("w2t", [HID, HID], f32, kind="ExternalInput")
    wft_t = nc.dram_tensor("wft", [HID, ENC], f32, kind="ExternalInput")
    b1bc_t = nc.dram_tensor("b1bc", [128, HID], f32, kind="ExternalInput")
    b2bc_t = nc.dram_tensor("b2bc", [128, HID], f32, kind="ExternalInput")
    bfbc_t = nc.dram_tensor("bfbc", [128, ENC], f32, kind="ExternalInput")
    ident_t = nc.dram_tensor("ident", [128, 128], f32, kind="ExternalInput")
    out_t = nc.dram_tensor("out", [SHARD_PAD, ENC], f32, kind="ExternalOutput")

    # per-subtile block lists: blocks[t] = ordered [(c, k), ...]
    blocks = [[(c, k) for c in range(NCHUNK) for k in range(int(nb[c][t]))]
              for t in range(NSUB)]

    with tile.TileContext(nc) as tc:
        with tc.tile_pool(name="const", bufs=1) as cst, \
             tc.tile_pool(name="edata", bufs=1) as edata, \
             tc.tile_pool(name="msgp", bufs=2) as msgp, \
             tc.tile_pool(name="indp", bufs=4) as indp, \
             tc.tile_pool(name="accp", bufs=3, space="PSUM") as accp, \
             tc.tile_pool(name="epsp", bufs=3, space="PSUM") as epsp, \
             tc.tile_pool(name="work", bufs=3) as work, \
             tc.tile_pool(name="dram", bufs=1, space="DRAM") as dram:

            # ---- persistent SBUF data ----
            idx_sb = edata.tile([128, TOT // 16], mybir.dt.int16)
            nc.sync.dma_start(idx_sb[:], idx16_t[:])

            dinv_sb = cst.tile([128, NSUB], f32)
            w1t_sb = cst.tile([F, HID], f32)
            w2t_sb = cst.tile([HID, HID], f32)
            wft_sb = cst.tile([HID, ENC], f32)
            b1bc_sb = cst.tile([128, HID], f32)
            b2bc_sb = cst.tile([128, HID], f32)
            bfbc_sb = cst.tile([128, ENC], f32)
            ident_sb = cst.tile([128, 128], f32)
            for sb_, t_ in ((dinv_sb, dinv_t_t), (w1t_sb, w1t_t), (w2t_sb, w2t_t),
                            (wft_sb, wft_t), (b1bc_sb, b1bc_t), (b2bc_sb, b2bc_t),
                            (bfbc_sb, bfbc_t), (ident_sb, ident_t)):
                nc.sync.dma_start(sb_[:], t_[:])

            r1self_sb = edata.tile([128, NSUB * HID], f32)

            r1sh = dram.tile([SHARD_PAD, HID], bf16)
            r1full = dram.tile([XROWS, HID], bf16, addr_space="Shared")

            def aggregate_layer(src_dram, layer):
                """Gather + indicator-matmul aggregation + per-subtile epilogue.

                Block order is subtile-major so each subtile's PSUM
                accumulation group opens and closes before the next one
                starts (accumulation groups are bank-granular)."""
                for s in range(NSUP):
                    subs = list(range(s * SUPSZ, min((s + 1) * SUPSZ, NSUB)))
                    msgs = {}
                    starts = {}
                    for c in range(NCHUNK):
                        start_slot = int(offs[c * NSUB + subs[0]])
                        end_slot = int(offs[c * NSUB + subs[-1] + 1])
                        L = end_slot - start_slot
                        if L == 0:
                            continue
                        starts[c] = start_slot
                        msg = msgp.tile([128, L], bf16, tag=f"msg{c}", bufs=2)
                        msgs[c] = msg
                        nc.gpsimd.dma_gather(
                            msg[:].rearrange("p (b f) -> p b f", f=128),
                            src_dram[c * CHUNK:(c + 1) * CHUNK, :],
                            idx_sb[:, start_slot // 16:end_slot // 16],
                            L, L, 128, elem_step=F,
                            single_packet=False,
                            queue_num=c,
                        )

                    # ---- per-subtile accumulate + drain ----
                    for t in subs:
                        acc = accp.tile([128, 128], f32, tag="acc")
                        for c in range(NCHUNK):
                            nbk = int(nb[c][t])
                            if nbk == 0:
                                continue
                            base = int(offs[c * NSUB + t])
                            ind = indp.tile([128, nbk * 128], bf16, tag="ind")
                            nc.scalar.dma_start(
                                ind[:], indb_t[:, base:base + nbk * 128])
                            for k in range(nbk):
                                mloc = (base - starts[c]) // 128 + k
                                nc.tensor.matmul(
                                    acc[:],
                                    lhsT=ind[:, k * 128:(k + 1) * 128],
                                    rhs=msgs[c][:, mloc * 128:(mloc + 1) * 128],
                                    start=(blocks[t][0] == (c, k)),
                                    stop=(blocks[t][-1] == (c, k)),
                                )

                        sum_sb = work.tile([128, F], f32, tag="sum")
                        if layer == 0:
                            self_tl = work.tile([128, F], f32, tag="selftl")
                            nc.sync.dma_start(
                                self_tl[:], xp_self_t[t * 128:(t + 1) * 128, :])
                        else:
                            self_tl = r1self_sb[:, t * HID:(t + 1) * HID]
                        if blocks[t]:
                            nc.vector.tensor_tensor(
                                out=sum_sb[:], in0=acc[:], in1=self_tl[:],
                                op=mybir.AluOpType.add)
                        else:
                            nc.vector.tensor_copy(out=sum_sb[:], in_=self_tl[:])

                        diag = work.tile([128, 128], f32, tag="diag")
                        nc.scalar.activation(
                            diag[:], ident_sb[:],
                            mybir.ActivationFunctionType.Copy,
                            scale=dinv_sb[:, t:t + 1])
                        tp = epsp.tile([128, 128], f32, tag="eps")
                        nc.tensor.matmul(tp[:], lhsT=sum_sb[:], rhs=diag[:],
                                         start=True, stop=True)
                        ts = work.tile([128, 128], f32, tag="ts")
                        nc.scalar.activation(ts[:], tp[:],
                                             mybir.ActivationFunctionType.Copy)

                        wsb = w1t_sb if layer == 0 else w2t_sb
                        op_ = epsp.tile([128, HID], f32, tag="eps")
                        nc.tensor.matmul(op_[:], lhsT=ts[:], rhs=wsb[:],
                                         start=True, stop=True)
                        z = work.tile([128, HID], f32, tag="z")
                        bbc = b1bc_sb if layer == 0 else b2bc_sb
                        nc.vector.tensor_tensor(out=z[:], in0=op_[:], in1=bbc[:],
                                                op=mybir.AluOpType.add)

                        if layer == 0:
                            # r1' = relu(z * dinv) kept f32 in SBUF + bf16 to HBM
                            nc.scalar.activation(
                                r1self_sb[:, t * HID:(t + 1) * HID], z[:],
                                mybir.ActivationFunctionType.Relu,
                                scale=dinv_sb[:, t:t + 1])
                            r1bf = work.tile([128, HID], bf16, tag="r1bf")
                            nc.scalar.activation(
                                r1bf[:], z[:],
                                mybir.ActivationFunctionType.Relu,
                                scale=dinv_sb[:, t:t + 1])
                            nc.sync.dma_start(
                                r1sh[t * 128:(t + 1) * 128, :], r1bf[:])
                        else:
                            r2 = work.tile([128, HID], f32, tag="r2")
                            nc.scalar.activation(
                                r2[:], z[:], mybir.ActivationFunctionType.Relu)
                            rtp = epsp.tile([128, 128], f32, tag="eps")
                            nc.tensor.matmul(rtp[:], lhsT=r2[:], rhs=ident_sb[:],
                                             start=True, stop=True)
                            rts = work.tile([128, 128], f32, tag="rts")
                            nc.scalar.activation(rts[:], rtp[:],
                                                 mybir.ActivationFunctionType.Copy)
                            fp = epsp.tile([128, ENC], f32, tag="eps")
                            nc.tensor.matmul(fp[:], lhsT=rts[:], rhs=wft_sb[:],
                                             start=True, stop=True)
                            fz = work.tile([128, ENC], f32, tag="fz")
                            nc.vector.tensor_tensor(out=fz[:], in0=fp[:],
                                                    in1=bfbc_sb[:],
                                                    op=mybir.AluOpType.add)
                            nc.sync.dma_start(
                                out_t[t * 128:(t + 1) * 128, :], fz[:])

            aggregate_layer(xp_bf_t, layer=0)
            if stage >= 2:
                nc.gpsimd.collective_compute(
                    "AllGather",
                    mybir.AluOpType.bypass,
                    replica_groups=[list(range(NCORES))],
                    ins=[r1sh[:].opt()],
                    outs=[r1full[:].opt()],
                )
            if stage >= 3:
                aggregate_layer(r1full, layer=1)
            else:
                for t in range(NSUB):
                    dbg = work.tile([128, ENC], f32, tag="fz")
                    nc.vector.tensor_copy(
                        out=dbg[:], in_=r1self_sb[:, t * HID:t * HID + ENC])
                    nc.sync.dma_start(out_t[t * 128:(t + 1) * 128, :], dbg[:])

    nc.compile()
    return nc


def kernel(**inputs):
    shared, per_core, nb, offs, TOT = _preprocess(
        inputs["x"], inputs["edge_index"], inputs["edge_weight"],
        inputs["W1"], inputs["b1"], inputs["W2"], inputs["b2"],
        inputs["Wf"], inputs["bf"])

    key = (TOT, nb.tobytes())
    if key not in _cache:
        _cache[key] = _build(nb, offs, TOT)
    nc = _cache[key]

    in_maps = []
    for d in range(NCORES):
        m = dict(shared)
        m.update(per_core[d])
        in_maps.append(m)

    res = bass_utils.run_bass_kernel_spmd(nc, in_maps, core_ids=list(range(NCORES)))
    out = np.concatenate(
        [res.results[d]["out"][:SHARD] for d in range(NCORES)], axis=0)
    return out.astype(np.float32)



# revision 5
# speedup vs baseline: 2.1854x; 1.3188x over previous
"""2-layer weighted-GCN embedding kernel for 8 Trainium2 NeuronCores.

Strategy (dst-sharded message passing, transposed dataflow):
  - Nodes are sharded by destination across the 8 cores (12500 each, padded
    to 12544 = 98 * 128).  Each core handles every edge whose dst lands in
    its shard, so the scatter-add is purely local.
  - GCN associativity: conv(x) = (A_hat @ x) @ W^T + b, so we aggregate RAW
    features first and apply the dense transform on the (sharded) aggregate.
  - Self-loops are regular edges; the full symmetric normalization
    dinv[src] * w * dinv[dst] is folded into the host-built indicator.
  - Scatter-add is an indicator matmul with the message block STATIONARY:
    acc[f, dst] = sum_slots msg[slot, f] * ind[slot, dst] comes out
    feature-partitioned, so the dense W matmul consumes it directly
    (lhsT = W^T) with no transpose in between.
  - Layer-0 messages x[src[slot]] are a compile-time permutation of the
    input, pre-gathered on the host and streamed sequentially (HWDGE).
    Layer-1 messages are gathered per-edge from the AllGather'd hidden
    state with SWDGE dma_gather, spread across all 4 SWDGE queues for
    4x parallel descriptor emission.
  - Between the two conv layers one AllGather shares r1 = relu(conv1)
    across cores (bf16).

kernel(**inputs) takes the FULL inputs and returns the FULL [100000, 64]
output; everything (sharding, compile, SPMD run, gather of shards) happens
inside.
"""

import numpy as np
import ml_dtypes

import concourse.bass as bass
import concourse.tile as tile
import concourse.bacc as bacc
from concourse import mybir, bass_utils

BF16 = ml_dtypes.bfloat16

F = 128
HID = 128
ENC = 64
NCORES = 8
SUBW = 128
SUPSZ = 4                      # subtiles per supertile (one gather covers these)


def _set_dims(n):
    """(Re)compute the node-count-derived global dims."""
    global N, SHARD, NSUB, SHARD_PAD, CHUNK, XROWS, NSUP
    N = n
    SHARD = N // NCORES
    NSUB = -(-SHARD // SUBW)           # subtiles per shard
    SHARD_PAD = NSUB * SUBW
    CHUNK = 2 * SHARD_PAD              # rows per gather chunk (< 2**15)
    XROWS = NCORES * SHARD_PAD         # padded node-table rows
    NSUP = -(-NSUB // SUPSZ)


NCHUNK = 4
_set_dims(100000)

_cache = {}


def _preprocess(x, edge_index, edge_weight, W1, b1, W2, b2, Wf, bf):
    """All host-side numpy prep: normalization, edge partitioning, layouts."""
    src = np.asarray(edge_index[0], dtype=np.int64)
    dst = np.asarray(edge_index[1], dtype=np.int64)
    w = np.asarray(edge_weight, dtype=np.float32)
    x = np.asarray(x, dtype=np.float32)

    deg = np.bincount(dst, weights=w.astype(np.float64), minlength=N) + 1.0
    dinv = (1.0 / np.sqrt(deg)).astype(np.float32)

    # self-loops as regular edges, full norm on every edge
    loop = np.arange(N, dtype=np.int64)
    src_f = np.concatenate([src, loop])
    dst_f = np.concatenate([dst, loop])
    nrm_f = np.concatenate([dinv[src] * w * dinv[dst], dinv * dinv])

    x_pad = np.zeros((XROWS, F), np.float32)
    for o in range(NCORES):
        x_pad[o * SHARD_PAD:o * SHARD_PAD + SHARD] = x[o * SHARD:(o + 1) * SHARD]
    x_bf = x_pad.astype(BF16)

    # map src node id -> (chunk, local row) in the padded table
    owner = src_f // SHARD
    src_pad = owner * SHARD_PAD + (src_f - owner * SHARD)
    chunk = src_pad // CHUNK
    src_local = (src_pad - chunk * CHUNK).astype(np.int64)
    assert src_local.max() < 2 ** 15

    NCELL = NCHUNK * NSUB  # flat cell id = c * NSUB + t

    # per-device cell contents
    dev = []
    counts = np.zeros((NCORES, NCELL), np.int64)
    for d in range(NCORES):
        lo, hi = d * SHARD, (d + 1) * SHARD
        m = (dst_f >= lo) & (dst_f < hi)
        dl = dst_f[m] - lo
        t = dl // SUBW
        cid = chunk[m] * NSUB + t
        order = np.argsort(cid, kind="stable")
        cid_s = cid[order]
        counts[d] = np.bincount(cid_s, minlength=NCELL)
        dev.append((cid_s,
                    src_local[m][order].astype(np.int16),
                    (dl % SUBW)[order].astype(np.int64),
                    nrm_f[m][order]))

    nb_cell = -(-counts.max(axis=0) // 128)            # blocks per cell (shared)
    cell_off = np.zeros(NCELL + 1, np.int64)
    np.cumsum(nb_cell * 128, out=cell_off[1:])
    TOT = int(cell_off[-1])

    per_core = []
    for d in range(NCORES):
        cid_s, sl, dr, nr = dev[d]
        starts = np.zeros(NCELL + 1, np.int64)
        np.cumsum(counts[d], out=starts[1:])
        rank = np.arange(len(cid_s)) - starts[cid_s]
        pos = cell_off[cid_s] + rank
        f_src = np.zeros(TOT, np.int16)
        f_dst = np.zeros(TOT, np.int64)
        f_nrm = np.zeros(TOT, np.float32)
        f_src[pos] = sl
        f_dst[pos] = dr
        f_nrm[pos] = nr
        f_chunk = np.zeros(TOT, np.int64)
        f_chunk[pos] = cid_s // NSUB

        idx16 = np.ascontiguousarray(np.tile(f_src.reshape(-1, 16).T, (8, 1)))

        # host-built indicators, partition-major:
        # indb[slot%128, (slot//128)*128 + dst_rel] = norm
        indb = np.zeros((128, TOT), BF16)
        posa = np.arange(TOT)
        indb[posa % 128, (posa // 128) * 128 + f_dst] = f_nrm.astype(BF16)

        # host pre-gathered layer-0 messages, same [128, TOT] layout as the
        # SBUF msg tiles: msg0[slot%128, (slot//128)*128 + f] = x[src[slot], f]
        gl = f_chunk * CHUNK + f_src.astype(np.int64)
        rows = x_bf[gl]                                  # [TOT, F]
        msg0 = np.ascontiguousarray(
            rows.reshape(TOT // 128, 128, F).transpose(1, 0, 2).reshape(128, TOT))

        per_core.append({
            "idx16": idx16,
            "indb": indb,
            "msg0": msg0,
        })

    shared = {
        "w1t": np.ascontiguousarray(np.asarray(W1, np.float32).T.astype(BF16)),
        "w2t": np.ascontiguousarray(np.asarray(W2, np.float32).T.astype(BF16)),
        "wft": np.ascontiguousarray(np.asarray(Wf, np.float32).T.astype(BF16)),
        "b1col": np.asarray(b1, np.float32).reshape(128, 1).copy(),
        "b2col": np.asarray(b2, np.float32).reshape(128, 1).copy(),
        "bfbc": np.broadcast_to(np.asarray(bf, np.float32), (128, ENC)).copy(),
        "identb": np.eye(128, dtype=np.float32).astype(BF16),
    }
    nb = nb_cell.reshape(NCHUNK, NSUB)      # [c][t]
    offs = cell_off.reshape(-1)             # flat slot offsets, id = c*NSUB+t
    return shared, per_core, nb, offs, TOT


def _build(nb, offs, TOT):
    """Build the SPMD bass program (identical for all 8 cores)."""
    nc = bacc.Bacc("TRN2", target_bir_lowering=False, debug=False,
                   num_devices=NCORES, num_swdge_queues=4)
    f32 = mybir.dt.float32
    bf16 = mybir.dt.bfloat16

    idx16_t = nc.dram_tensor("idx16", [128, TOT // 16], mybir.dt.int16, kind="ExternalInput")
    indb_t = nc.dram_tensor("indb", [128, TOT], bf16, kind="ExternalInput")
    msg0_t = nc.dram_tensor("msg0", [128, TOT], bf16, kind="ExternalInput")
    w1t_t = nc.dram_tensor("w1t", [F, HID], bf16, kind="ExternalInput")
    w2t_t = nc.dram_tensor("w2t", [HID, HID], bf16, kind="ExternalInput")
    wft_t = nc.dram_tensor("wft", [HID, ENC], bf16, kind="ExternalInput")
    b1col_t = nc.dram_tensor("b1col", [128, 1], f32, kind="ExternalInput")
    b2col_t = nc.dram_tensor("b2col", [128, 1], f32, kind="ExternalInput")
    bfbc_t = nc.dram_tensor("bfbc", [128, ENC], f32, kind="ExternalInput")
    identb_t = nc.dram_tensor("identb", [128, 128], bf16, kind="ExternalInput")
    out_t = nc.dram_tensor("out", [SHARD_PAD, ENC], f32, kind="ExternalOutput")

    # per-subtile block lists: blocks[t] = ordered [(c, k), ...]
    blocks = [[(c, k) for c in range(NCHUNK) for k in range(int(nb[c][t]))]
              for t in range(NSUB)]

    with tile.TileContext(nc) as tc:
        with tc.tile_pool(name="const", bufs=1) as cst, \
             tc.tile_pool(name="edata", bufs=1) as edata, \
             tc.tile_pool(name="msgp", bufs=3) as msgp, \
             tc.tile_pool(name="indp", bufs=3) as indp, \
             tc.tile_pool(name="accp", bufs=3, space="PSUM") as accp, \
             tc.tile_pool(name="epsp", bufs=3, space="PSUM") as epsp, \
             tc.tile_pool(name="tpsp", bufs=2, space="PSUM") as tpsp, \
             tc.tile_pool(name="work", bufs=3) as work, \
             tc.tile_pool(name="dram", bufs=1, space="DRAM") as dram:

            # ---- persistent SBUF data ----
            idx_sb = edata.tile([128, TOT // 16], mybir.dt.int16)
            nc.sync.dma_start(idx_sb[:], idx16_t[:])

            w1t_sb = cst.tile([F, HID], bf16)
            w2t_sb = cst.tile([HID, HID], bf16)
            wft_sb = cst.tile([HID, ENC], bf16)
            b1col_sb = cst.tile([128, 1], f32)
            b2col_sb = cst.tile([128, 1], f32)
            bfbc_sb = cst.tile([128, ENC], f32)
            ident_sb = cst.tile([128, 128], bf16)
            for sb_, t_ in ((w1t_sb, w1t_t), (w2t_sb, w2t_t), (wft_sb, wft_t),
                            (b1col_sb, b1col_t), (b2col_sb, b2col_t),
                            (bfbc_sb, bfbc_t), (ident_sb, identb_t)):
                nc.sync.dma_start(sb_[:], t_[:])

            r1sh = dram.tile([SHARD_PAD, HID], bf16)
            r1full = dram.tile([XROWS, HID], bf16, addr_space="Shared")

            def aggregate_layer(layer):
                """Messages + indicator -> transposed aggregate -> dense.

                layer 0: messages streamed from host-pregathered msg0.
                layer 1: messages gathered per-edge from r1full."""
                for s in range(NSUP):
                    subs = list(range(s * SUPSZ, min((s + 1) * SUPSZ, NSUB)))
                    msgs = {}
                    inds = {}
                    starts = {}
                    for c in range(NCHUNK):
                        start_slot = int(offs[c * NSUB + subs[0]])
                        end_slot = int(offs[c * NSUB + subs[-1] + 1])
                        L = end_slot - start_slot
                        if L == 0:
                            continue
                        starts[c] = start_slot
                        msg = msgp.tile([128, L], bf16, tag=f"msg{c}")
                        msgs[c] = msg
                        if layer == 0:
                            nc.sync.dma_start(
                                msg[:], msg0_t[:, start_slot:end_slot])
                        else:
                            nc.gpsimd.dma_gather(
                                msg[:].rearrange("p (b f) -> p b f", f=128),
                                r1full[c * CHUNK:(c + 1) * CHUNK, :],
                                idx_sb[:, start_slot // 16:end_slot // 16],
                                L, L, 128, elem_step=F,
                                single_packet=False,
                                queue_num=c,
                            )
                        ind = indp.tile([128, L], bf16, tag=f"ind{c}")
                        inds[c] = ind
                        nc.scalar.dma_start(
                            ind[:], indb_t[:, start_slot:end_slot])

                    # ---- per-subtile accumulate + epilogue ----
                    for t in subs:
                        acc = accp.tile([128, 128], f32, tag="acc")
                        for c in range(NCHUNK):
                            nbk = int(nb[c][t])
                            if nbk == 0:
                                continue
                            base = int(offs[c * NSUB + t]) - starts[c]
                            for k in range(nbk):
                                o = base + k * 128
                                nc.tensor.matmul(
                                    acc[:],
                                    lhsT=msgs[c][:, o:o + 128],
                                    rhs=inds[c][:, o:o + 128],
                                    start=(blocks[t][0] == (c, k)),
                                    stop=(blocks[t][-1] == (c, k)),
                                )

                        # sum over slots now sits as acc[f, dst] in PSUM
                        sum_sb = work.tile([128, 128], bf16, tag="sum")
                        nc.vector.tensor_copy(out=sum_sb[:], in_=acc[:])

                        if layer == 0:
                            z1 = epsp.tile([128, 128], f32, tag="eps")
                            nc.tensor.matmul(z1[:], lhsT=w1t_sb[:], rhs=sum_sb[:],
                                             start=True, stop=True)
                            r1t = work.tile([128, 128], bf16, tag="r1t")
                            nc.scalar.activation(
                                r1t[:], z1[:],
                                mybir.ActivationFunctionType.Relu,
                                bias=b1col_sb[:, 0:1])
                            rp = tpsp.tile([128, 128], bf16, tag="tp")
                            nc.tensor.transpose(rp[:], r1t[:], ident_sb[:])
                            r1 = work.tile([128, HID], bf16, tag="r1")
                            nc.scalar.activation(
                                r1[:], rp[:],
                                mybir.ActivationFunctionType.Copy)
                            nc.sync.dma_start(
                                r1sh[t * 128:(t + 1) * 128, :], r1[:])
                        else:
                            z2 = epsp.tile([128, 128], f32, tag="eps")
                            nc.tensor.matmul(z2[:], lhsT=w2t_sb[:], rhs=sum_sb[:],
                                             start=True, stop=True)
                            r2t = work.tile([128, 128], bf16, tag="r1t")
                            nc.scalar.activation(
                                r2t[:], z2[:],
                                mybir.ActivationFunctionType.Relu,
                                bias=b2col_sb[:, 0:1])
                            fp = tpsp.tile([128, ENC], f32, tag="tp")
                            nc.tensor.matmul(fp[:], lhsT=r2t[:], rhs=wft_sb[:],
                                             start=True, stop=True)
                            fz = work.tile([128, ENC], f32, tag="fz")
                            nc.vector.tensor_tensor(out=fz[:], in0=fp[:],
                                                    in1=bfbc_sb[:],
                                                    op=mybir.AluOpType.add)
                            nc.sync.dma_start(
                                out_t[t * 128:(t + 1) * 128, :], fz[:])

            aggregate_layer(0)
            nc.gpsimd.collective_compute(
                "AllGather",
                mybir.AluOpType.bypass,
                replica_groups=[list(range(NCORES))],
                ins=[r1sh[:].opt()],
                outs=[r1full[:].opt()],
            )
            aggregate_layer(1)

    nc.compile()
    return nc


def kernel(**inputs):
    shared, per_core, nb, offs, TOT = _preprocess(
        inputs["x"], inputs["edge_index"], inputs["edge_weight"],
        inputs["W1"], inputs["b1"], inputs["W2"], inputs["b2"],
        inputs["Wf"], inputs["bf"])

    key = (TOT, nb.tobytes())
    if key not in _cache:
        _cache[key] = _build(nb, offs, TOT)
    nc = _cache[key]

    in_maps = []
    for d in range(NCORES):
        m = dict(shared)
        m.update(per_core[d])
        in_maps.append(m)

    res = bass_utils.run_bass_kernel_spmd(nc, in_maps, core_ids=list(range(NCORES)))
    out = np.concatenate(
        [res.results[d]["out"][:SHARD] for d in range(NCORES)], axis=0)
    return out.astype(np.float32)
